# revision 23
# baseline (speedup 1.0000x reference)
"""Dual-softmax cross-attention kernel for Trainium2 (Bass/Tile), 8 NeuronCores.

Problem: out = (0.3*softmax(q@kT) + 0.7*softmax(q2@kT)) @ v  projected by Wo + bo
  q  = x1 @ Wq, q2 = x2 @ Wq2, k = context @ Wk, v = context @ Wv
  shapes: x1/x2/context [4, 2048, 512]; 4 heads x 64 dim; out [4, 2048, 512].

Sharding: 8 cores = 4 batches x 2 query-halves. Each core computes the full
attention (all 4 heads) for its 1024 queries against the full 2048-key context
of its batch. No cross-core reductions needed; host concatenates outputs.

Per-core algorithm (all matmuls via fp32r = full-rate fp32 path on the PE,
HW-measured ~1 cyc/row for moving dims >= 256):
  - transpose x1/x2/ctx to [512, seq] via PE-transpose (contraction dim must
    live on partitions); projections qT/q2T ([256, seq] as 2 pair tiles) and
    v natural [seq, 256] via lhsT=ctxT
  - kTz: per-head zero-padded [128, m] tiles (head's 64 dh rows live, other 64
    rows zero) so each scores^T matmul is a plain K=128 MM; measured much
    faster in-stream than K=64 tile_position row-packing with fp32r
  - per j-tile: both heads' scores go to one 2-bank PSUM tile and a single
    fused ACT exp ([128, 2, 512], scale folded) produces e^T; the AV matmuls
    trail 3 j-tiles behind (software pipeline)
  - U = [v | 1].T @ e  (M=65 fused matmul: rows 0..63 = (attn@v)^T
    un-normalized, row 64 = softmax denominator Z), accumulated in PSUM
  - blend 0.3*U1/Z1 + 0.7*U2/Z2: reciprocal of the Z row in place, broadcast
    across partitions via a DRAM bounce (DMA), fused multiply via
    scalar_tensor_tensor; blend/out-projection are emitted one query-chunk
    late so they overlap the next attention block
  - out = oT.T @ Wo + bo (4 accumulating K=64 matmuls; bias pre-broadcast)
"""

import numpy as np

import concourse.bacc as bacc
import concourse.mybir as mybir
import concourse.tile as tile
from concourse.masks import make_identity

F32 = mybir.dt.float32
F32R = mybir.dt.float32r

B, N, M_CTX = 4, 2048, 2048
C = 512        # query/context dim
H = 4          # heads
DH = 64        # dim per head
INNER = H * DH  # 256
E = 512        # output dim
SCALE = DH ** -0.5
N_CORES = 8
N_I = N // 2   # queries per core


def r(ap):
    """Bitcast an f32 AP to float32r for full-rate PE consumption."""
    return ap.bitcast(F32R)


def build_attention_nc(n_i=N_I, m=M_CTX, reps=1, variant="full", qk_bf16=True):
    """Build the per-core Bass program. reps>1 wraps the body in a hardware
    loop that recomputes the same outputs (used for wall-clock timing)."""
    n_ct = C // 128            # contraction tiles for the projections
    n_jt = m // 128            # key tiles
    ich = min(512, n_i)        # query chunk (free dim of most matmuls)
    n_ic = n_i // ich
    n_kt_o = INNER // 128      # contraction tiles for the out-projection

    nc = bacc.Bacc("TRN2", target_bir_lowering=False, debug=False,
                   num_devices=N_CORES)
    x1h = nc.declare_dram_parameter("x1h", [n_i, C], F32, isOutput=False)
    x2h = nc.declare_dram_parameter("x2h", [n_i, C], F32, isOutput=False)
    ctx = nc.declare_dram_parameter("ctx", [m, C], F32, isOutput=False)
    wq = nc.declare_dram_parameter("Wq", [C, INNER], F32, isOutput=False)
    wq2 = nc.declare_dram_parameter("Wq2", [C, INNER], F32, isOutput=False)
    wk = nc.declare_dram_parameter("Wk", [C, INNER], F32, isOutput=False)
    wv = nc.declare_dram_parameter("Wv", [C, INNER], F32, isOutput=False)
    wo = nc.declare_dram_parameter("Wo", [INNER, E], F32, isOutput=False)
    bo = nc.declare_dram_parameter("bo", [E], F32, isOutput=False)
    out = nc.declare_dram_parameter("out", [n_i, E], F32, isOutput=True)

    with tile.TileContext(nc) as tc:
        def body(ctx_stack):
            enter = ctx_stack.enter_context

            consts = enter(tc.tile_pool(name="consts", bufs=1))
            ident = consts.tile([128, 128], F32, tag="ident")
            make_identity(nc, ident)
            bo_bc = consts.tile([128, E], F32, tag="bo_bc")
            ones65 = consts.tile([65, 64], F32, tag="ones65")
            nc.vector.memset(ones65[:], 1.0)
            nc.sync.dma_start(out=bo_bc[:], in_=bo.ap().partition_broadcast(128))

            # ---- weights to SBUF ----
            w_sb = {}
            for name, w in (("wq", wq), ("wq2", wq2), ("wk", wk), ("wv", wv)):
                for ct in range(n_ct):
                    t = consts.tile([128, INNER], F32, tag=f"{name}{ct}")
                    nc.sync.dma_start(out=r(t[:]), in_=r(w[ct * 128:(ct + 1) * 128, :]))
                    w_sb[name, ct] = t
            wo_sb = []
            for h in range(H):
                t = consts.tile([64, E], F32, tag=f"wo{h}")
                nc.sync.dma_start(out=r(t[:]), in_=r(wo[h * 64:(h + 1) * 64, :]))
                wo_sb.append(t)

            # ---- persistent activations ----
            persist = enter(tc.tile_pool(name="persist", bufs=1))
            QKDT = mybir.dt.bfloat16 if qk_bf16 else F32
            q1T = [persist.tile([128, n_i], QKDT, tag=f"q1T{p}", name=f"q1T{p}") for p in range(2)]
            q2T = [persist.tile([128, n_i], QKDT, tag=f"q2T{p}", name=f"q2T{p}") for p in range(2)]
            if qk_bf16:
                # pair-stacked bf16 kT; heads sliced by partition range + packed
                kT = [persist.tile([128, m], QKDT, tag=f"kT{p}", name=f"kT{p}")
                      for p in range(2)]
            else:
                kTz = [[persist.tile([128, m], F32, tag=f"kTz{p}{h}", name=f"kTz{p}{h}")
                        for h in range(2)] for p in range(2)]
                for p in range(2):
                    for h in range(2):
                        nc.vector.memset(kTz[p][h][:], 0.0)
            vplus = [persist.tile([128, H, DH + 1], F32, tag=f"vp{jt}", name=f"vp{jt}")
                     for jt in range(n_jt)]

            # ================= phase A: transpose inputs =================
            # ================= phase B: projections =====================
            with tc.tile_pool(name="xT", bufs=1) as xt_pool, \
                 tc.tile_pool(name="xnat", bufs=8) as xnat_pool, \
                 tc.tile_pool(name="ps_a", bufs=2, space="PSUM") as ps_a:
                x1T = [xt_pool.tile([128, n_i], F32, tag=f"x1T{ct}", name=f"x1T{ct}") for ct in range(n_ct)]
                x2T = [xt_pool.tile([128, n_i], F32, tag=f"x2T{ct}", name=f"x2T{ct}") for ct in range(n_ct)]
                cT = [xt_pool.tile([128, m], F32, tag=f"cT{ct}", name=f"cT{ct}") for ct in range(n_ct)]

                for src, dstT, seq in ((x1h, x1T, n_i), (x2h, x2T, n_i), (ctx, cT, m)):
                    for ig in range(seq // 512):
                        nats = []
                        for k in range(4):
                            t = xnat_pool.tile([128, C], F32)
                            nc.sync.dma_start(
                                out=t[:],
                                in_=src[(ig * 4 + k) * 128:(ig * 4 + k + 1) * 128, :])
                            nats.append(t)
                        for ct in range(n_ct):
                            pt = ps_a.tile([128, 512], F32, tag="tr", bufs=3)
                            for k in range(4):
                                nc.tensor.transpose(
                                    pt[:, k * 128:(k + 1) * 128],
                                    nats[k][:, ct * 128:(ct + 1) * 128],
                                    ident[:])
                            dst = r(dstT[ct][:, ig * 512:(ig + 1) * 512])
                            if ct % 2 == 0:
                                nc.vector.tensor_copy(dst, pt[:])
                            else:
                                nc.scalar.copy(dst, pt[:])

                # q/q2/k projections: dst[p] = W[:, p*128:(p+1)*128].T @ xT
                for wname, srcT, dstT, seq in (
                        ("wq", x1T, q1T, n_i), ("wq2", x2T, q2T, n_i),
                        ("wk", cT, None, m)):
                    for p in range(2):
                        for ch in range(seq // 512):
                            pt = ps_a.tile([128, 512], F32, tag="proj", bufs=3)
                            for ct in range(n_ct):
                                nc.tensor.matmul(
                                    pt[:],
                                    r(w_sb[wname, ct][:, p * 128:(p + 1) * 128]),
                                    r(srcT[ct][:, ch * 512:(ch + 1) * 512]),
                                    start=(ct == 0), stop=(ct == n_ct - 1))
                            csl = slice(ch * 512, (ch + 1) * 512)
                            if wname == "wk":
                                if qk_bf16:
                                    dst = kT[p][:, csl]
                                    if (p + ch) % 2 == 0:
                                        nc.vector.tensor_copy(dst, pt[:])
                                    else:
                                        nc.scalar.copy(dst, pt[:])
                                else:
                                    for h in range(2):
                                        hs = slice(h * 64, (h + 1) * 64)
                                        dst = r(kTz[p][h][hs, csl])
                                        if h == 0:
                                            nc.vector.tensor_copy(dst, pt[hs, :])
                                        else:
                                            nc.scalar.copy(dst, pt[hs, :])
                            else:
                                dst = dstT[p][:, csl]
                                if not qk_bf16:
                                    dst = r(dst)
                                if (p + ch) % 2 == 0:
                                    nc.vector.tensor_copy(dst, pt[:])
                                else:
                                    nc.scalar.copy(dst, pt[:])

                # v projection (natural layout) + ones column for the Z row
                for jt in range(n_jt):
                    pv = ps_a.tile([128, INNER], F32, tag="vproj")
                    for ct in range(n_ct):
                        nc.tensor.matmul(
                            pv[:],
                            r(cT[ct][:, jt * 128:(jt + 1) * 128]),
                            r(w_sb["wv", ct][:]),
                            start=(ct == 0), stop=(ct == n_ct - 1))
                    nc.vector.memset(vplus[jt][:], 1.0)
                    for h in range(H):
                        dst = r(vplus[jt][:, h, 0:DH])
                        if h % 2 == 0:
                            nc.vector.tensor_copy(dst, pv[:, h * DH:(h + 1) * DH])
                        else:
                            nc.scalar.copy(dst, pv[:, h * DH:(h + 1) * DH])

            if variant == "AB":
                return
            # ================= phase C: attention =================
            with tc.tile_pool(name="sc", bufs=2, space="PSUM") as sc_pool, \
                 tc.tile_pool(name="upsum", bufs=2, space="PSUM") as u_pool, \
                 tc.tile_pool(name="eT", bufs=8) as e_pool, \
                 tc.tile_pool(name="usb", bufs=10) as usb_pool, \
                 tc.tile_pool(name="blend", bufs=6) as blend_pool, \
                 tc.tile_pool(name="oT", bufs=6) as o_pool, \
                 tc.tile_pool(name="osb", bufs=4) as osb_pool, \
                 tc.tile_pool(name="zdram", bufs=4, space="DRAM") as zdram_pool:
                tails = []

                def emit_blend(u_sb, oT):
                    for h2 in range(2):
                        oh = o_pool.tile([64, ich], F32, tag="oh", name="oh")
                        if variant == "noblend":
                            nc.vector.tensor_copy(r(oh[:]), u_sb[0, h2][0:DH, :])
                            oT.append(oh)
                            continue
                        tmp = blend_pool.tile([64, ich], F32, tag="bt", name="bt")
                        for s, coef in ((0, 0.3), (1, 0.7)):
                            # reciprocal of Z in place (partition 64), bounce
                            # through DRAM to broadcast across 64 partitions
                            nc.vector.reciprocal(u_sb[s, h2][DH:DH + 1, :],
                                                 u_sb[s, h2][DH:DH + 1, :])
                            zd = zdram_pool.tile([1, ich], F32, tag="zd", name="zd")
                            nc.sync.dma_start(out=zd[:],
                                              in_=u_sb[s, h2][DH:DH + 1, :])
                            rb = blend_pool.tile([64, ich], F32, tag="rb", name="rb")
                            nc.sync.dma_start(out=rb[:],
                                              in_=zd[:].partition_broadcast(64))
                            dst = tmp[:] if s == 0 else r(oh[:])
                            nc.vector.scalar_tensor_tensor(
                                dst, u_sb[s, h2][0:DH, :], coef, rb[:],
                                op0=mybir.AluOpType.mult,
                                op1=mybir.AluOpType.mult)
                        nc.vector.tensor_add(r(oh[:]), oh[:], tmp[:])
                        oT.append(oh)

                def emit_outproj(oT, ic):
                    for mt in range(ich // 128):
                        po = sc_pool.tile([128, E], F32, tag="tail", bufs=2, name="po")
                        for h in range(H):
                            nc.tensor.matmul(
                                po[:],
                                r(oT[h][:, mt * 128:(mt + 1) * 128]),
                                r(wo_sb[h][:]),
                                start=(h == 0), stop=(h == H - 1))
                        ob = osb_pool.tile([128, E], F32, name="ob")
                        nc.vector.tensor_add(ob[:], po[:], bo_bc[:])
                        nc.sync.dma_start(
                            out=out[ic * ich + mt * 128:ic * ich + (mt + 1) * 128, :],
                            in_=ob[:])

                for ic in range(n_ic):
                    isl = slice(ic * ich, (ic + 1) * ich)
                    oT = []
                    for p in range(2):
                        u_sb = {}
                        for s, qT in ((0, q1T), (1, q2T)):
                            u_ps = [u_pool.tile([DH + 1, ich], F32, name="u_ps") for _ in range(2)]
                            DELAY = 3
                            ets = {}
                            for step in range(n_jt + DELAY):
                                if step < n_jt:
                                    jt = step
                                    jsl = slice(jt * 128, (jt + 1) * 128)
                                    sc = sc_pool.tile([128, 2, ich], F32, tag="sc")
                                    for h2 in range(2):
                                        if qk_bf16:
                                            psl = slice(h2 * 64, (h2 + 1) * 64)
                                            nc.tensor.matmul(
                                                sc[:, h2, :], kT[p][psl, jsl],
                                                qT[p][psl, isl],
                                                start=True, stop=True,
                                                tile_position=(h2 * 64, 0))
                                        else:
                                            nc.tensor.matmul(
                                                sc[:, h2, :], r(kTz[p][h2][:, jsl]),
                                                r(qT[p][:, isl]),
                                                start=True, stop=True)
                                    et = e_pool.tile([128, 2, ich], F32)
                                    nc.scalar.activation(
                                        r(et[:]), sc[:],
                                        mybir.ActivationFunctionType.Exp,
                                        scale=SCALE)
                                    ets[jt] = et
                                if step >= DELAY and variant != "ABQ":
                                    jt = step - DELAY
                                    et = ets.pop(jt)
                                    for h2 in range(2):
                                        nc.tensor.matmul(
                                            u_ps[h2][:],
                                            r(vplus[jt][:, 2 * p + h2, :]),
                                            r(et[:, h2, :]),
                                            start=(jt == 0), stop=(jt == n_jt - 1))
                            if variant == "ABQ":
                                continue
                            for h2 in range(2):
                                ut = usb_pool.tile([DH + 1, ich], F32)
                                nc.vector.tensor_copy(ut[:], u_ps[h2][:])
                                u_sb[s, h2] = ut
                        if variant in ("ABQ", "ABQA"):
                            continue
                        tails.append(lambda u_sb=u_sb, oT=oT: emit_blend(u_sb, oT))

                    if variant in ("ABQ", "ABQA"):
                        continue
                    tails.append(lambda oT=oT, ic=ic: emit_outproj(oT, ic))
                    # run deferred tails one attention block behind
                    while len(tails) > 3:
                        tails.pop(0)()
                for fn in tails:
                    fn()

        from contextlib import ExitStack
        if reps == 1:
            with ExitStack() as st:
                body(st)
        else:
            with tc.For_i(0, reps, 1):
                with ExitStack() as st:
                    body(st)

    nc.compile()
    return nc


_NC_CACHE = {}


def _get_nc():
    if "nc" not in _NC_CACHE:
        _NC_CACHE["nc"] = build_attention_nc()
    return _NC_CACHE["nc"]


def kernel(x1, x2, context, Wq, Wq2, Wk, Wv, Wo, bo):
    from concourse.bass_utils import run_bass_kernel_spmd

    nc = _get_nc()
    x1 = np.ascontiguousarray(np.asarray(x1, dtype=np.float32))
    x2 = np.ascontiguousarray(np.asarray(x2, dtype=np.float32))
    context = np.ascontiguousarray(np.asarray(context, dtype=np.float32))
    shared = {
        "Wq": np.ascontiguousarray(np.asarray(Wq, np.float32)),
        "Wq2": np.ascontiguousarray(np.asarray(Wq2, np.float32)),
        "Wk": np.ascontiguousarray(np.asarray(Wk, np.float32)),
        "Wv": np.ascontiguousarray(np.asarray(Wv, np.float32)),
        "Wo": np.ascontiguousarray(np.asarray(Wo, np.float32)),
        "bo": np.ascontiguousarray(np.asarray(bo, np.float32)),
    }
    in_maps = []
    for core in range(N_CORES):
        b, half = divmod(core, 2)
        qsl = slice(half * N_I, (half + 1) * N_I)
        in_maps.append({
            "x1h": np.ascontiguousarray(x1[b, qsl]),
            "x2h": np.ascontiguousarray(x2[b, qsl]),
            "ctx": np.ascontiguousarray(context[b]),
            **shared,
        })
    res = run_bass_kernel_spmd(nc, in_maps, core_ids=list(range(N_CORES)))
    full = np.empty((B, N, E), dtype=np.float32)
    for core in range(N_CORES):
        b, half = divmod(core, 2)
        full[b, half * N_I:(half + 1) * N_I] = res.results[core]["out"]
    return full
